# revision 38
# baseline (speedup 1.0000x reference)
"""Trainium2 Bass kernel for nn_CrossAttention_31791347925417.

Math (per batch b, per stream tok in {x, blood} with weight W in {W1, W2}):
    kv = tok @ W.T ; k, v heads [H, N, D]
    ctx = softmax_d( SCALE * k_h^T v_h )          # [H, D, D], softmax over first D
    out_x = x_h @ ctx2_h ; out_b = blood_h @ ctx1_h

Gram trick: k_h^T v_h = W_k_h (tok^T tok) W_v_h^T with G = tok^T tok [C, C], so
the N=4096 contraction happens once per stream; everything downstream is tiny
[C,C]-scale work.  ctx probs are written into block-diagonal BD tiles used by
the output matmuls out[n, (h,e)] = sum_{(h,d)} tokT[(h,d), n] * BD[(h,d), (h,e)].

This version is fully fp16 on-chip (validated ~2e-3 rel err vs the 2e-2 gate):
  - Host supplies tokens BOTH natural [N, C] (for G) and pre-transposed [C, N]
    (for the output matmuls) in fp16, so there are no on-chip token transposes
    and no PSUM->SBUF cast traffic for them.
  - All matmuls run at 1 cycle/row (fp16); G uses the true upper triangle
    (G_OFF = [0,128,256,384]) and mirrors the 6 lower blocks via PE transposes.
  - Outputs are written fp16 and upcast on the host.

Sharding: data-parallel over batch B=8 across the 8 cores; weights replicated.
Host pre-transposes W -> W.T [C, 2C], folds SCALE into the k-half (exact 2^-3),
and casts to fp16.
"""

import sys

if "/opt/trn_rl_repo" not in sys.path:
    sys.path.insert(0, "/opt/trn_rl_repo")

import numpy as np

from concourse import bacc, masks, mybir, tile
from concourse.bass_utils import run_bass_kernel_spmd

B, N, C, H = 8, 4096, 512, 8
D = C // H
SCALE = D ** -0.5
P = 128
NBIG = N // 512          # 8 big row tiles (512 rows each)
NT = N // P              # 32 n-tiles
CB = C // P              # 4 column blocks == head pairs
F32 = mybir.dt.float32
F16 = mybir.dt.float16
AX = mybir.AxisListType
ACT_EXP = mybir.ActivationFunctionType.Exp

# G row-block m computes columns [G_OFF[m], C) -- true upper triangle.
G_OFF = [0, P, 2 * P, 3 * P]


def build_nc():
    nc = bacc.Bacc("TRN2", target_bir_lowering=False, debug=False)

    # host-packed SBUF-layout inputs: tokens [NBIG, P, 4C], xT [C, N],
    # weights [P, CB*2C] -- all fully contiguous per partition row.
    xb = nc.dram_tensor("xb", [NBIG, P, 4 * C], F16, kind="ExternalInput").ap()
    bb = nc.dram_tensor("bb", [NBIG, P, 4 * C], F16, kind="ExternalInput").ap()
    xbt = nc.dram_tensor("xbt", [C, N], F16, kind="ExternalInput").ap()
    bbt = nc.dram_tensor("bbt", [C, N], F16, kind="ExternalInput").ap()
    w1t = nc.dram_tensor("w1t", [P, CB * 2 * C], F16, kind="ExternalInput").ap()
    w2t = nc.dram_tensor("w2t", [P, CB * 2 * C], F16, kind="ExternalInput").ap()
    # blocked transposed output layout: [kb, part(c within pair), pair, n-col]
    ox = nc.dram_tensor("oxT", [NBIG, P, CB, 512], F16, kind="ExternalOutput").ap()
    ob = nc.dram_tensor("obT", [NBIG, P, CB, 512], F16, kind="ExternalOutput").ap()

    with tile.TileContext(nc) as tc:
        _emit(nc, tc, xb, bb, xbt, bbt, w1t, w2t, ox, ob)

    nc.compile()
    return nc


def _emit(nc, tc, xb, bb, xbt, bbt, w1t, w2t, ox, ob):
    from contextlib import ExitStack

    ctx = ExitStack()
    with ctx:
        const = ctx.enter_context(tc.tile_pool(name="const", bufs=1))
        wpool = ctx.enter_context(tc.tile_pool(name="wpool", bufs=1))
        tokp = ctx.enter_context(tc.tile_pool(name="tokp", bufs=1))
        xtp = ctx.enter_context(tc.tile_pool(name="xtp", bufs=1))
        gqp = ctx.enter_context(tc.tile_pool(name="gqp", bufs=16))
        smallp = ctx.enter_context(tc.tile_pool(name="smallp", bufs=2))
        fpool = ctx.enter_context(tc.tile_pool(name="fpool", bufs=2))
        bdpool = ctx.enter_context(tc.tile_pool(name="bdpool", bufs=8))
        ostp = ctx.enter_context(tc.tile_pool(name="ostp", bufs=4))
        psA = ctx.enter_context(tc.tile_pool(name="psA", bufs=4, space="PSUM"))
        psB = ctx.enter_context(tc.tile_pool(name="psB", bufs=4, space="PSUM"))

        ident = const.tile([P, P], F16, tag="idh")
        masks.make_identity(nc, ident[:])

        # weights: chunk j (c-rows 128j..128j+128) lives at cols [j*2C, (j+1)*2C)
        w_x = wpool.tile([P, CB * 2 * C], F16, tag="wx")
        w_b = wpool.tile([P, CB * 2 * C], F16, tag="wb")

        def load_weights():
            # w dram is host-packed [P, CB*2C]: contiguous per partition.
            nc.sync.dma_start(w_x[:], w1t[:])
            nc.sync.dma_start(w_b[:], w2t[:])

        def wchunk(w, j):
            return w[:, j * 2 * C:(j + 1) * 2 * C]

        # transposed tokens (from host): pair block m at cols [m*N, (m+1)*N)
        xT_x = xtp.tile([P, CB * N], F16, tag="xtx")
        xT_b = xtp.tile([P, CB * N], F16, tag="xtb")

        def load_xT(xT, tdram, n0=0, n1=N):
            # one dma_start per n-range of the [C, N] transposed stream (the
            # n-range maps to out chunks kb = n0/512..n1/512, so halves can
            # be prioritized independently); 4KB+ descriptors.
            nc.sync.dma_start(
                xT[:].rearrange("p (m n) -> p m n", m=CB)[:, :, n0:n1],
                tdram[:, n0:n1].rearrange("(m p) n -> p m n", p=P),
            )

        def load_tok_group(tok_dram, pfx, kb0, nkb, split=False, per_kb=False):
            """One SBUF tile for big tiles kb0..kb0+nkb-1.  per_kb issues one
            dma_start per big tile (fine-grained completion for PE pacing);
            otherwise a single dma_start covers the group (one completion)."""
            tg = tokp.tile([P, nkb * 4 * C], F16, tag=f"{pfx}{kb0}", name=f"{pfx}{kb0}")
            if split:
                for sub in range(nkb * 4):
                    nc.sync.dma_start(
                        tg[:, sub * C:(sub + 1) * C],
                        tok_dram[kb0][:, sub * C:(sub + 1) * C],
                    )
            elif per_kb:
                for i in range(nkb):
                    nc.sync.dma_start(
                        tg[:, i * 4 * C:(i + 1) * 4 * C], tok_dram[kb0 + i]
                    )
            else:
                nc.sync.dma_start(
                    tg[:].rearrange("p (k c) -> p k c", k=nkb),
                    tok_dram[kb0:kb0 + nkb],
                )
            return [tg[:, i * 4 * C:(i + 1) * 4 * C] for i in range(nkb)]

        def emit_G_tile(gps, sb, k):
            for m in range(CB):
                o = G_OFF[m]
                nc.tensor.matmul(
                    gps[m][:, o:C], sb[:, m * P:(m + 1) * P], sb[:, o:C],
                    start=(k == 0), stop=(k == NT - 1),
                )

        def drain_G(gps, engs):
            """PSUM G (f32) -> SBUF fp16 row-block tiles."""
            g_sb = []
            for m in range(CB):
                o = G_OFF[m]
                g = gqp.tile([P, C], F16, tag="gq", name=f"g{m}")
                engs[m % len(engs)](g[:, o:C], gps[m][:, o:C])
                g_sb.append(g)
            return g_sb

        def emit_mirrors(g_sb, pspool):
            """Fill missing lower blocks (i,j), j < G_OFF[i]//P, from (j,i)^T."""
            for i in range(CB):
                for j in range(G_OFF[i] // P):
                    mps = pspool.tile([P, P], F16, tag="g", name="mps")
                    nc.tensor.transpose(
                        mps[:], g_sb[j][:, i * P:(i + 1) * P], ident[:]
                    )
                    nc.vector.tensor_copy(g_sb[i][:, j * P:(j + 1) * P], mps[:])

        def emit_Q_ctx(g_sb, w, pspool, mid=None, qeng=None):
            """Q = G @ Wk, then per-pair ctxT psum blocks (f32, kept in PSUM).
            `mid` emits PE filler between Q and ctx (covers q-drain latency)."""
            q_sb = [None] * CB
            for i in reversed(range(CB)):
                qp = pspool.tile([P, C], F32, tag="g", name=f"qp{i}")
                for j in range(CB):
                    nc.tensor.matmul(
                        qp[:], g_sb[j][:, i * P:(i + 1) * P],
                        wchunk(w, j)[:, 0:C], start=(j == 0), stop=(j == 3),
                    )
                q = gqp.tile([P, C], F16, tag="gq", name=f"q{i}")
                # chain-critical; engine choice depends on ACT backlog
                (qeng or [nc.vector.tensor_copy, nc.scalar.copy])[i % len(
                    qeng or [0, 0])](q[:], qp[:])
                q_sb[i] = q
            if mid is not None:
                mid()
            cps_l = []
            for p in range(CB):
                cps = pspool.tile([P, P], F32, tag="g", name=f"cps{p}")
                for j in range(CB):
                    nc.tensor.matmul(
                        cps[:],
                        wchunk(w, j)[:, C + p * P:C + (p + 1) * P],
                        q_sb[j][:, p * P:(p + 1) * P],
                        start=(j == 0), stop=(j == 3),
                    )
                cps_l.append(cps)
            return cps_l

        def emit_softmax(cps_l):
            """softmax over d (free axis) of each diagonal 64x64 block; returns
            fp16 F tiles (block-diag probs, not yet transposed)."""
            fps = []
            for p in range(CB):
                cps = cps_l[p]
                nm = smallp.tile([P, 1], F32, tag="nm", name="nm")
                sm = smallp.tile([P, 1], F32, tag="sm", name="sm")
                rv = smallp.tile([P, 1], F32, tag="rv", name="rv")
                pp = smallp.tile([P, D], F32, tag="pp", name="pp")
                fp = fpool.tile([P, P], F16, tag="F", name="fp")
                nc.gpsimd.memset(fp[:], 0.0)
                for dd in range(2):
                    s0 = slice(dd * D, (dd + 1) * D)
                    blk = cps[s0, s0]
                    nc.vector.reduce_max(nm[s0, :], blk, axis=AX.X, negate=True)
                    nc.scalar.activation(
                        pp[s0, :], blk, ACT_EXP, bias=nm[s0, :], scale=1.0,
                        accum_out=sm[s0, :],
                    )
                nc.vector.reciprocal(rv[:], sm[:])
                for dd in range(2):
                    s0 = slice(dd * D, (dd + 1) * D)
                    nc.vector.tensor_scalar_mul(fp[s0, s0], pp[s0, :], rv[s0, :])
                fps.append(fp)
            return fps

        def emit_BD(fps, pspool):
            """PE-transpose prob tiles into block-diagonal ctx operands."""
            BDs = []
            for p in range(CB):
                bps = pspool.tile([P, P], F16, tag="g", name="bps")
                nc.tensor.transpose(bps[:], fps[p][:], ident[:])
                bd = bdpool.tile([P, P], F16, tag="bd", name=f"bd{p}")
                nc.vector.tensor_copy(bd[:], bps[:])
                BDs.append(bd)
            return BDs

        def out_chunk(xT, BDs, kb, odram, pool, drains, hot=False):
            """outT for 512 n-cols (tile-group kb): per pair p one matmul
            [c-block p, 512 n]; drain into ost quarter p on `drains` engines.
            `hot` prepends a dummy transpose into the (start=True-reset) psum
            so the drain-paced PE idle doesn't drop the p-state clock."""
            ost = ostp.tile([P, 4 * 512], F16, tag="ost", name="ost")
            for p in range(CB):
                ops = pool.tile([P, 512], F32, tag="g", name=f"ops{p}")
                nc.tensor.matmul(
                    ops[:], BDs[p][:], xT[:, p * N + kb * 512:p * N + (kb + 1) * 512],
                    start=True, stop=True,
                )
                drains[p % len(drains)](ost[:, p * 512:(p + 1) * 512], ops[:])
            # one full-ost DMA: 4KB/partition descriptors run at full DMA
            # rate; issued on the sync ring (idle once inputs are issued)
            nc.sync.dma_start(
                odram[kb],
                ost[:].rearrange("part (q n) -> part q n", q=CB),
            )

        ACT3_DVE1 = [nc.scalar.copy, nc.scalar.copy, nc.vector.tensor_copy,
                     nc.vector.tensor_copy]
        ACT_DVE = [nc.scalar.copy, nc.vector.tensor_copy]

        # ---- emission schedule ----
        # DMA completion order == issue order on the sync ring; 12 input
        # dma_starts all fit the ring depth so there are no issue stalls.
        # Weights must land before chain_x (~32us), xT_b before the fused
        # G_b/out_b phase (~42us), tok_b groups pace the fused phase, xT_x
        # by ~60us.
        toks_x = load_tok_group(xb, "tx", 0, 1, split=True)
        toks_x += load_tok_group(xb, "tx", 1, NBIG - 1, per_kb=True)
        toks_b = load_tok_group(bb, "tb", 0, 2)
        load_weights()
        toks_b += load_tok_group(bb, "tb", 2, 3)
        load_xT(xT_b, bbt, 0, N // 2)      # covers out_b chunks 0..3
        toks_b += load_tok_group(bb, "tb", 5, 3)
        load_xT(xT_b, bbt, N // 2, N)      # covers out_b chunks 4..7
        load_xT(xT_x, xbt)

        def emit_G_big(gps, toks, kb):
            for sub in range(4):
                emit_G_tile(gps, toks[kb][:, sub * C:(sub + 1) * C], kb * 4 + sub)

        # G_x: 32 k-tiles, 4 psum banks.  Warmup transposes keep the PE busy
        # from t~2us so its p-state ramps to max clock before real work; they
        # scribble on gps_x[0], which G kb0 (start=True) resets anyway.
        gps_x = [psA.tile([P, C], F32, tag="g", name=f"gpsx{m}") for m in range(CB)]
        warm = gps_x[0][:, 0:P // 2].bitcast(F16)
        for _ in range(38):
            nc.tensor.transpose(warm, ident[:], ident[:])
        for kb in range(NBIG):
            emit_G_big(gps_x, toks_x, kb)
        g_sb_x = drain_G(gps_x, [nc.vector.tensor_copy])  # DVE, runs during G_b[0]

        # G_b tile 0 covers the g_x drain latency on PE
        gps_b = [psB.tile([P, C], F32, tag="g", name=f"gpsb{m}") for m in range(CB)]
        emit_G_big(gps_b, toks_b, 0)

        # chain x PE part: mirrors, Q, ctx (psA banks freed by g_x drains)
        emit_mirrors(g_sb_x, psA)
        cps_x = emit_Q_ctx(g_sb_x, w_x, psA)
        fps_x = emit_softmax(cps_x)                   # DVE/ACT

        # G_b tile 1 covers softmax_x latency on PE
        emit_G_big(gps_b, toks_b, 1)
        bd1 = emit_BD(fps_x, psA)                     # ctx1 (from x) -> out_b

        # G_b tiles 2..7 interleaved with out_b chunks 0..3: out drains go
        # 3:1 ACT:DVE and hide under the G_b matmuls; DVE keeps slack for the
        # chain-critical copies.  Chunks 4..7 fill chain_b's latency pockets.
        for kb in range(2, NBIG):
            emit_G_big(gps_b, toks_b, kb)
            if kb - 2 < 4:
                out_chunk(xT_b, bd1, kb - 2, ob, psA, ACT3_DVE1)
        g_sb_b = drain_G(gps_b, [nc.vector.tensor_copy, nc.scalar.copy])  # both free now

        # chain b runs with BOTH drain engines free (fused drains caught up
        # during bare tiles 6/7); out_b chunks 4..7 cover softmax/BD latency
        emit_mirrors(g_sb_b, psB)
        cps_b = emit_Q_ctx(g_sb_b, w_b, psB)
        fps_b = emit_softmax(cps_b)
        out_chunk(xT_b, bd1, 4, ob, psA, ACT3_DVE1, hot=True)
        out_chunk(xT_b, bd1, 5, ob, psA, ACT3_DVE1, hot=True)
        out_chunk(xT_b, bd1, 6, ob, psA, ACT3_DVE1, hot=True)
        bd2 = emit_BD(fps_b, psB)                      # ctx2 (from blood) -> out_x
        out_chunk(xT_b, bd1, 7, ob, psA, ACT3_DVE1, hot=True)

        for kb in range(NBIG):
            out_chunk(xT_x, bd2, kb, ox, psB if kb % 2 == 0 else psA,
                      ACT_DVE, hot=True)


_NC_CACHE = None


def _get_nc():
    global _NC_CACHE
    if _NC_CACHE is None:
        _NC_CACHE = build_nc()
    return _NC_CACHE


def _prep_inputs(x, blood, W1, W2):
    x = np.asarray(x, dtype=np.float32)
    blood = np.asarray(blood, dtype=np.float32)
    w1t = np.asarray(W1, dtype=np.float32).T.copy()
    w2t = np.asarray(W2, dtype=np.float32).T.copy()
    w1t[:, :C] *= SCALE  # fold softmax scale into the k-projection (exact: 2^-3)
    w2t[:, :C] *= SCALE
    x16 = x.astype(np.float16)
    b16 = blood.astype(np.float16)

    def pack_tok(t):
        # [N, C] -> [NBIG, P, 4C] with tokb[kb][p, s*C + c] = t[kb*512+s*128+p, c]
        return np.ascontiguousarray(
            t.reshape(NBIG, 4, P, C).transpose(0, 2, 1, 3).reshape(NBIG, P, 4 * C))

    def pack_w(wt):
        # [C, 2C] -> [P, CB*2C] with w[p, j*2C + c] = wt[j*128+p, c]
        return np.ascontiguousarray(
            wt.reshape(CB, P, 2 * C).transpose(1, 0, 2).reshape(P, CB * 2 * C))

    w1p = pack_w(w1t.astype(np.float16))
    w2p = pack_w(w2t.astype(np.float16))
    return [
        {
            "xb": pack_tok(x16[b]),
            "bb": pack_tok(b16[b]),
            "xbt": np.ascontiguousarray(x16[b].T),
            "bbt": np.ascontiguousarray(b16[b].T),
            "w1t": w1p,
            "w2t": w2p,
        }
        for b in range(B)
    ]


def _unshuffle(arr):
    """[NBIG, P, CB, 512] blocked-transposed fp16 -> [N, C] f32 natural."""
    # arr[kb, part, p, col] = out[kb*512 + col, p*128 + part]
    return np.ascontiguousarray(
        arr.transpose(0, 3, 2, 1).reshape(N, C).astype(np.float32))


def kernel(x, blood, W1, W2, trace=False):
    nc = _get_nc()
    in_maps = _prep_inputs(x, blood, W1, W2)
    res = run_bass_kernel_spmd(nc, in_maps, core_ids=list(range(B)), trace=trace)
    out_x = np.stack([_unshuffle(res.results[b]["oxT"]) for b in range(B)])
    out_b = np.stack([_unshuffle(res.results[b]["obT"]) for b in range(B)])
    if trace:
        kernel.last_results = res
    return (out_x, out_b)


# revision 39
# speedup vs baseline: 1.0418x; 1.0418x over previous
"""Trainium2 Bass kernel for nn_CrossAttention_31791347925417.

Math (per batch b, per stream tok in {x, blood} with weight W in {W1, W2}):
    kv = tok @ W.T ; k, v heads [H, N, D]
    ctx = softmax_d( SCALE * k_h^T v_h )          # [H, D, D], softmax over first D
    out_x = x_h @ ctx2_h ; out_b = blood_h @ ctx1_h

Gram trick: k_h^T v_h = W_k_h (tok^T tok) W_v_h^T with G = tok^T tok [C, C], so
the N=4096 contraction happens once per stream; everything downstream is tiny
[C,C]-scale work.  ctx probs are written into block-diagonal BD tiles used by
the output matmuls out[n, (h,e)] = sum_{(h,d)} tokT[(h,d), n] * BD[(h,d), (h,e)].

This version is fully fp16 on-chip (validated ~2e-3 rel err vs the 2e-2 gate):
  - Host supplies tokens BOTH natural [N, C] (for G) and pre-transposed [C, N]
    (for the output matmuls) in fp16, so there are no on-chip token transposes
    and no PSUM->SBUF cast traffic for them.
  - All matmuls run at 1 cycle/row (fp16); G uses the true upper triangle
    (G_OFF = [0,128,256,384]) and mirrors the 6 lower blocks via PE transposes.
  - Outputs are written fp16 and upcast on the host.

Sharding: data-parallel over batch B=8 across the 8 cores; weights replicated.
Host pre-transposes W -> W.T [C, 2C], folds SCALE into the k-half (exact 2^-3),
and casts to fp16.
"""

import sys

if "/opt/trn_rl_repo" not in sys.path:
    sys.path.insert(0, "/opt/trn_rl_repo")

import numpy as np

from concourse import bacc, masks, mybir, tile
from concourse.bass_utils import run_bass_kernel_spmd

B, N, C, H = 8, 4096, 512, 8
D = C // H
SCALE = D ** -0.5
P = 128
NBIG = N // 512          # 8 big row tiles (512 rows each)
NT = N // P              # 32 n-tiles
CB = C // P              # 4 column blocks == head pairs
F32 = mybir.dt.float32
F16 = mybir.dt.float16
AX = mybir.AxisListType
ACT_EXP = mybir.ActivationFunctionType.Exp

# G row-block m computes columns [G_OFF[m], C) -- true upper triangle.
G_OFF = [0, P, 2 * P, 3 * P]


def build_nc():
    nc = bacc.Bacc("TRN2", target_bir_lowering=False, debug=False)

    # host-packed SBUF-layout inputs: tokens [NBIG, P, 4C], xT [C, N],
    # weights [P, CB*2C] -- all fully contiguous per partition row.
    xb = nc.dram_tensor("xb", [NBIG, P, 4 * C], F16, kind="ExternalInput").ap()
    bb = nc.dram_tensor("bb", [NBIG, P, 4 * C], F16, kind="ExternalInput").ap()
    xbt = nc.dram_tensor("xbt", [C, N], F16, kind="ExternalInput").ap()
    bbt = nc.dram_tensor("bbt", [C, N], F16, kind="ExternalInput").ap()
    w1t = nc.dram_tensor("w1t", [P, CB * 2 * C], F16, kind="ExternalInput").ap()
    w2t = nc.dram_tensor("w2t", [P, CB * 2 * C], F16, kind="ExternalInput").ap()
    # blocked transposed output layout: [kb, part(c within pair), pair, n-col]
    ox = nc.dram_tensor("oxT", [NBIG, P, CB, 512], F16, kind="ExternalOutput").ap()
    ob = nc.dram_tensor("obT", [NBIG, P, CB, 512], F16, kind="ExternalOutput").ap()

    with tile.TileContext(nc) as tc:
        _emit(nc, tc, xb, bb, xbt, bbt, w1t, w2t, ox, ob)

    nc.compile()
    return nc


def _emit(nc, tc, xb, bb, xbt, bbt, w1t, w2t, ox, ob):
    from contextlib import ExitStack

    ctx = ExitStack()
    with ctx:
        const = ctx.enter_context(tc.tile_pool(name="const", bufs=1))
        wpool = ctx.enter_context(tc.tile_pool(name="wpool", bufs=1))
        tokp = ctx.enter_context(tc.tile_pool(name="tokp", bufs=1))
        xtp = ctx.enter_context(tc.tile_pool(name="xtp", bufs=1))
        gqp = ctx.enter_context(tc.tile_pool(name="gqp", bufs=16))
        smallp = ctx.enter_context(tc.tile_pool(name="smallp", bufs=2))
        fpool = ctx.enter_context(tc.tile_pool(name="fpool", bufs=2))
        bdpool = ctx.enter_context(tc.tile_pool(name="bdpool", bufs=8))
        ostp = ctx.enter_context(tc.tile_pool(name="ostp", bufs=6))
        psA = ctx.enter_context(tc.tile_pool(name="psA", bufs=4, space="PSUM"))
        psB = ctx.enter_context(tc.tile_pool(name="psB", bufs=4, space="PSUM"))

        ident = const.tile([P, P], F16, tag="idh")
        masks.make_identity(nc, ident[:])

        # weights: chunk j (c-rows 128j..128j+128) lives at cols [j*2C, (j+1)*2C)
        w_x = wpool.tile([P, CB * 2 * C], F16, tag="wx")
        w_b = wpool.tile([P, CB * 2 * C], F16, tag="wb")

        def load_weights():
            # w dram is host-packed [P, CB*2C]: contiguous per partition.
            nc.sync.dma_start(w_x[:], w1t[:])
            nc.sync.dma_start(w_b[:], w2t[:])

        def wchunk(w, j):
            return w[:, j * 2 * C:(j + 1) * 2 * C]

        # transposed tokens (from host): pair block m at cols [m*N, (m+1)*N)
        xT_x = xtp.tile([P, CB * N], F16, tag="xtx")
        xT_b = xtp.tile([P, CB * N], F16, tag="xtb")

        def load_xT(xT, tdram, n0=0, n1=N):
            # one dma_start per n-range of the [C, N] transposed stream (the
            # n-range maps to out chunks kb = n0/512..n1/512, so halves can
            # be prioritized independently); 4KB+ descriptors.
            nc.sync.dma_start(
                xT[:].rearrange("p (m n) -> p m n", m=CB)[:, :, n0:n1],
                tdram[:, n0:n1].rearrange("(m p) n -> p m n", p=P),
            )

        def load_tok_group(tok_dram, pfx, kb0, nkb, split=False, per_kb=False):
            """One SBUF tile for big tiles kb0..kb0+nkb-1.  per_kb issues one
            dma_start per big tile (fine-grained completion for PE pacing);
            otherwise a single dma_start covers the group (one completion)."""
            tg = tokp.tile([P, nkb * 4 * C], F16, tag=f"{pfx}{kb0}", name=f"{pfx}{kb0}")
            if split:
                for sub in range(nkb * 4):
                    nc.sync.dma_start(
                        tg[:, sub * C:(sub + 1) * C],
                        tok_dram[kb0][:, sub * C:(sub + 1) * C],
                    )
            elif per_kb:
                for i in range(nkb):
                    nc.sync.dma_start(
                        tg[:, i * 4 * C:(i + 1) * 4 * C], tok_dram[kb0 + i]
                    )
            else:
                nc.sync.dma_start(
                    tg[:].rearrange("p (k c) -> p k c", k=nkb),
                    tok_dram[kb0:kb0 + nkb],
                )
            return [tg[:, i * 4 * C:(i + 1) * 4 * C] for i in range(nkb)]

        def emit_G_tile(gps, sb, k):
            for m in range(CB):
                o = G_OFF[m]
                nc.tensor.matmul(
                    gps[m][:, o:C], sb[:, m * P:(m + 1) * P], sb[:, o:C],
                    start=(k == 0), stop=(k == NT - 1),
                )

        def drain_G(gps, engs):
            """PSUM G (f32) -> SBUF fp16 row-block tiles."""
            g_sb = []
            for m in range(CB):
                o = G_OFF[m]
                g = gqp.tile([P, C], F16, tag="gq", name=f"g{m}")
                engs[m % len(engs)](g[:, o:C], gps[m][:, o:C])
                g_sb.append(g)
            return g_sb

        def emit_mirrors(g_sb, pspool):
            """Fill missing lower blocks (i,j), j < G_OFF[i]//P, from (j,i)^T."""
            for i in range(CB):
                for j in range(G_OFF[i] // P):
                    mps = pspool.tile([P, P], F16, tag="g", name="mps")
                    nc.tensor.transpose(
                        mps[:], g_sb[j][:, i * P:(i + 1) * P], ident[:]
                    )
                    nc.vector.tensor_copy(g_sb[i][:, j * P:(j + 1) * P], mps[:])

        def emit_Q_ctx(g_sb, w, pspool, mid=None, qeng=None):
            """Q = G @ Wk, then per-pair ctxT psum blocks (f32, kept in PSUM).
            `mid` emits PE filler between Q and ctx (covers q-drain latency)."""
            q_sb = [None] * CB
            for i in reversed(range(CB)):
                qp = pspool.tile([P, C], F32, tag="g", name=f"qp{i}")
                for j in range(CB):
                    nc.tensor.matmul(
                        qp[:], g_sb[j][:, i * P:(i + 1) * P],
                        wchunk(w, j)[:, 0:C], start=(j == 0), stop=(j == 3),
                    )
                q = gqp.tile([P, C], F16, tag="gq", name=f"q{i}")
                # chain-critical; engine choice depends on ACT backlog
                (qeng or [nc.vector.tensor_copy, nc.scalar.copy])[i % len(
                    qeng or [0, 0])](q[:], qp[:])
                q_sb[i] = q
            if mid is not None:
                mid()
            cps_l = []
            for p in range(CB):
                cps = pspool.tile([P, P], F32, tag="g", name=f"cps{p}")
                for j in range(CB):
                    nc.tensor.matmul(
                        cps[:],
                        wchunk(w, j)[:, C + p * P:C + (p + 1) * P],
                        q_sb[j][:, p * P:(p + 1) * P],
                        start=(j == 0), stop=(j == 3),
                    )
                cps_l.append(cps)
            return cps_l

        def emit_softmax(cps_l):
            """softmax over d (free axis) of each diagonal 64x64 block; returns
            fp16 F tiles (block-diag probs, not yet transposed)."""
            fps = []
            for p in range(CB):
                cps = cps_l[p]
                nm = smallp.tile([P, 1], F32, tag="nm", name="nm")
                sm = smallp.tile([P, 1], F32, tag="sm", name="sm")
                rv = smallp.tile([P, 1], F32, tag="rv", name="rv")
                pp = smallp.tile([P, D], F32, tag="pp", name="pp")
                fp = fpool.tile([P, P], F16, tag="F", name="fp")
                nc.gpsimd.memset(fp[:], 0.0)
                for dd in range(2):
                    s0 = slice(dd * D, (dd + 1) * D)
                    blk = cps[s0, s0]
                    nc.vector.reduce_max(nm[s0, :], blk, axis=AX.X, negate=True)
                    nc.scalar.activation(
                        pp[s0, :], blk, ACT_EXP, bias=nm[s0, :], scale=1.0,
                        accum_out=sm[s0, :],
                    )
                nc.vector.reciprocal(rv[:], sm[:])
                for dd in range(2):
                    s0 = slice(dd * D, (dd + 1) * D)
                    nc.vector.tensor_scalar_mul(fp[s0, s0], pp[s0, :], rv[s0, :])
                fps.append(fp)
            return fps

        def emit_BD(fps, pspool):
            """PE-transpose prob tiles into block-diagonal ctx operands."""
            BDs = []
            for p in range(CB):
                bps = pspool.tile([P, P], F16, tag="g", name="bps")
                nc.tensor.transpose(bps[:], fps[p][:], ident[:])
                bd = bdpool.tile([P, P], F16, tag="bd", name=f"bd{p}")
                nc.vector.tensor_copy(bd[:], bps[:])
                BDs.append(bd)
            return BDs

        def out_chunk(xT, BDs, kb, odram, pool, drains, hot=False):
            """outT for 512 n-cols (tile-group kb): per pair p one matmul
            [c-block p, 512 n]; drain into ost quarter p on `drains` engines.
            `hot` prepends a dummy transpose into the (start=True-reset) psum
            so the drain-paced PE idle doesn't drop the p-state clock."""
            ost = ostp.tile([P, 4 * 512], F16, tag="ost", name="ost")
            for p in range(CB):
                ops = pool.tile([P, 512], F32, tag="g", name=f"ops{p}")
                nc.tensor.matmul(
                    ops[:], BDs[p][:], xT[:, p * N + kb * 512:p * N + (kb + 1) * 512],
                    start=True, stop=True,
                )
                drains[p % len(drains)](ost[:, p * 512:(p + 1) * 512], ops[:])
            # one full-ost DMA: 4KB/partition descriptors run at full DMA
            # rate; issued on the sync ring (idle once inputs are issued)
            nc.sync.dma_start(
                odram[kb],
                ost[:].rearrange("part (q n) -> part q n", q=CB),
            )

        ACT3_DVE1 = [nc.scalar.copy, nc.scalar.copy, nc.vector.tensor_copy,
                     nc.vector.tensor_copy]
        ACT_DVE = [nc.scalar.copy, nc.vector.tensor_copy]

        # ---- emission schedule ----
        # DMA completion order == issue order on the sync ring; 12 input
        # dma_starts all fit the ring depth so there are no issue stalls.
        # Weights must land before chain_x (~32us), xT_b before the fused
        # G_b/out_b phase (~42us), tok_b groups pace the fused phase, xT_x
        # by ~60us.
        toks_x = load_tok_group(xb, "tx", 0, 1, split=True)
        toks_x += load_tok_group(xb, "tx", 1, NBIG - 1, per_kb=True)
        toks_b = load_tok_group(bb, "tb", 0, 2)
        load_weights()
        toks_b += load_tok_group(bb, "tb", 2, 3)
        load_xT(xT_b, bbt, 0, N // 2)      # covers out_b chunks 0..3
        toks_b += load_tok_group(bb, "tb", 5, 3)
        load_xT(xT_b, bbt, N // 2, N)      # covers out_b chunks 4..7
        load_xT(xT_x, xbt)

        def emit_G_big(gps, toks, kb):
            for sub in range(4):
                emit_G_tile(gps, toks[kb][:, sub * C:(sub + 1) * C], kb * 4 + sub)

        # G_x: 32 k-tiles, 4 psum banks.  Warmup transposes keep the PE busy
        # from t~2us so its p-state ramps to max clock before real work; they
        # scribble on gps_x[0], which G kb0 (start=True) resets anyway.
        gps_x = [psA.tile([P, C], F32, tag="g", name=f"gpsx{m}") for m in range(CB)]
        warm = gps_x[0][:, 0:P // 2].bitcast(F16)
        for _ in range(38):
            nc.tensor.transpose(warm, ident[:], ident[:])
        for kb in range(NBIG):
            emit_G_big(gps_x, toks_x, kb)
        g_sb_x = drain_G(gps_x, [nc.vector.tensor_copy, nc.scalar.copy])  # both idle

        # G_b tile 0 covers the g_x drain latency on PE
        gps_b = [psB.tile([P, C], F32, tag="g", name=f"gpsb{m}") for m in range(CB)]
        emit_G_big(gps_b, toks_b, 0)

        # chain x PE part: mirrors, Q (tile1 fills the q-drain pocket), ctx
        emit_mirrors(g_sb_x, psA)
        cps_x = emit_Q_ctx(g_sb_x, w_x, psA,
                           mid=lambda: emit_G_big(gps_b, toks_b, 1))
        fps_x = emit_softmax(cps_x)                   # DVE/ACT

        # G_b tile 2 covers softmax_x latency on PE
        emit_G_big(gps_b, toks_b, 2)
        bd1 = emit_BD(fps_x, psA)                     # ctx1 (from x) -> out_b

        # G_b tiles 3..7 interleaved with out_b chunks 0..3 (tile 7 bare so
        # drains catch up); drains 2:2 ACT:DVE hide under the G_b matmuls.
        for kb in range(3, NBIG):
            emit_G_big(gps_b, toks_b, kb)
            if kb - 3 < 4:
                out_chunk(xT_b, bd1, kb - 3, ob, psA, ACT3_DVE1)
        g_sb_b = drain_G(gps_b, [nc.vector.tensor_copy, nc.scalar.copy])  # both free now

        # chain b runs with BOTH drain engines free (fused drains caught up
        # during bare tiles 6/7); out_b chunks 4..7 cover softmax/BD latency
        emit_mirrors(g_sb_b, psB)
        cps_b = emit_Q_ctx(g_sb_b, w_b, psB)
        fps_b = emit_softmax(cps_b)
        out_chunk(xT_b, bd1, 4, ob, psA, ACT3_DVE1, hot=True)
        out_chunk(xT_b, bd1, 5, ob, psA, ACT3_DVE1, hot=True)
        out_chunk(xT_b, bd1, 6, ob, psA, ACT3_DVE1, hot=True)
        bd2 = emit_BD(fps_b, psB)                      # ctx2 (from blood) -> out_x
        out_chunk(xT_b, bd1, 7, ob, psA, ACT3_DVE1, hot=True)

        for kb in range(NBIG):
            out_chunk(xT_x, bd2, kb, ox, psB if kb % 2 == 0 else psA,
                      ACT_DVE, hot=True)


_NC_CACHE = None


def _get_nc():
    global _NC_CACHE
    if _NC_CACHE is None:
        _NC_CACHE = build_nc()
    return _NC_CACHE


def _prep_inputs(x, blood, W1, W2):
    x = np.asarray(x, dtype=np.float32)
    blood = np.asarray(blood, dtype=np.float32)
    w1t = np.asarray(W1, dtype=np.float32).T.copy()
    w2t = np.asarray(W2, dtype=np.float32).T.copy()
    w1t[:, :C] *= SCALE  # fold softmax scale into the k-projection (exact: 2^-3)
    w2t[:, :C] *= SCALE
    x16 = x.astype(np.float16)
    b16 = blood.astype(np.float16)

    def pack_tok(t):
        # [N, C] -> [NBIG, P, 4C] with tokb[kb][p, s*C + c] = t[kb*512+s*128+p, c]
        return np.ascontiguousarray(
            t.reshape(NBIG, 4, P, C).transpose(0, 2, 1, 3).reshape(NBIG, P, 4 * C))

    def pack_w(wt):
        # [C, 2C] -> [P, CB*2C] with w[p, j*2C + c] = wt[j*128+p, c]
        return np.ascontiguousarray(
            wt.reshape(CB, P, 2 * C).transpose(1, 0, 2).reshape(P, CB * 2 * C))

    w1p = pack_w(w1t.astype(np.float16))
    w2p = pack_w(w2t.astype(np.float16))
    return [
        {
            "xb": pack_tok(x16[b]),
            "bb": pack_tok(b16[b]),
            "xbt": np.ascontiguousarray(x16[b].T),
            "bbt": np.ascontiguousarray(b16[b].T),
            "w1t": w1p,
            "w2t": w2p,
        }
        for b in range(B)
    ]


def _unshuffle(arr):
    """[NBIG, P, CB, 512] blocked-transposed fp16 -> [N, C] f32 natural."""
    # arr[kb, part, p, col] = out[kb*512 + col, p*128 + part]
    return np.ascontiguousarray(
        arr.transpose(0, 3, 2, 1).reshape(N, C).astype(np.float32))


def kernel(x, blood, W1, W2, trace=False):
    nc = _get_nc()
    in_maps = _prep_inputs(x, blood, W1, W2)
    res = run_bass_kernel_spmd(nc, in_maps, core_ids=list(range(B)), trace=trace)
    out_x = np.stack([_unshuffle(res.results[b]["oxT"]) for b in range(B)])
    out_b = np.stack([_unshuffle(res.results[b]["obT"]) for b in range(B)])
    if trace:
        kernel.last_results = res
    return (out_x, out_b)
